# revision 11
# baseline (speedup 1.0000x reference)
"""Trainium2 Bass kernel for masked attention (nn_Attention_77704548319841).

Reference computation per batch b:
    CI     = einsum('sc,hc->hs', context[b], W_a)          # [H, S]
    scores = einsum('th,hs->ts', dec[b], CI)               # [T, S]
    scores = where(mask, -1e6, scores)
    attn   = softmax(scores, axis=-1)
    out[b] = einsum('ts,sc->tc', attn, context[b])         # [T, 2H]

Sharding: pure data parallel over batch (16 batches / 8 cores = 2 per core).
W_a is replicated.

Per-core pipeline (matmuls on TensorE, fp16 operands, f32 PSUM accum):
  mm1: CI[h,s]     = W_aT.T @ ctxT        (lhsT = W_a.T  [C,H], rhs = ctx.T [C,S])
  mm2: scores[t,s] = decT.T @ CI          (lhsT = dec.T  [H,T], rhs = CI    [H,S])
  softmax (free-dim S):  masked = mask*-1e6 + scores   (one DVE scalar_tensor_tensor)
                         exp    = Exp(masked - SHIFT), rowsum via ACT accum_out
                         attn   = exp * (1/rowsum)     (fp16, values in [0,1])
  mm3: out[t,c]    = attnT.T @ ctx        (lhsT = attn.T [S,T], rhs = ctx [S,C])

PE order: startup transposes + mm1(0), softmax-mm(0), mm1(1), mm3(0),
softmax-mm(1), mm3(1).  This order makes the sync-queue transposes'
readiness order equal their need order (ctxT b1 -> attnT b0 -> decT b1 ->
attnT b1), which matters because the tile scheduler orders queue work by
readiness.

Measured platform facts this schedule is built around:
  - plain big DMA ~300GB/s per queue; HBM ~358GB/s per core aggregate
  - HWDGE queues (scalar/sync) BLOCK THE ISSUING ENGINE until ring credits
    free (~1 outstanding big DMA); the gpsimd SWDGE queue does not block
  - xbar transpose ~1.27us per call regardless of size; ALL transposes must
    stay on ONE queue (concurrent transposes from two queues corrupt data);
    explicit dep-chains between them add ~1.8us each - rely on natural
    queue serialization instead
  - Pool-engine (gpsimd) casts are ~6x slower than DVE casts - never use
  - PE p-state: long idle gaps are followed by a half-speed ramp window

Queue plan:
  gpsimd SWDGE : ctx b0 ch0/ch1, ctx b1 ch0/ch1, dec b1, mask b1 (the
                 urgent stream + WAR-gated mask; non-blocking issues),
                 then odd-mt output stores
  scalar queue : wa, dec b0, mask b0 (issued while ACT idle), even stores
  sync queue   : ALL xbar transposes: ctxT b1, attnT b0, decT b1, attnT b1
  TensorE      : matmuls + startup transposes of W_a/ctx-b0/dec-b0 (PE idle)
  VectorE      : f32->fp16 casts, psum evictions, masking, attn scale (half)
  ScalarE      : exp (+fused rowsum), attn scale (half), half mm3 evictions

Output is stored as fp16 (halves store traffic; ~1e-4 extra rel err) and
upcast to f32 on the host after the gather.

Softmax normalization is mathematically exact w.r.t. the reference: a constant
shift (instead of rowmax) leaves softmax unchanged; masked entries get
exp(s - 1e6 - SHIFT) == 0, identical to the reference's where(-1e6) after its
own exp underflow.
"""

import numpy as np
from contextlib import ExitStack

import concourse.bass as bass
import concourse.tile as tile
from concourse import bacc, mybir
from concourse.masks import make_identity
from concourse.bass_utils import run_bass_kernel_spmd

B, T, S, H = 16, 1024, 1024, 512
C = 2 * H
N_CORES = 8
BLOC = B // N_CORES  # batches per core
P = 128
NT = T // P   # 8 t-tiles
NS = S // P   # 8 s-tiles
NH = H // P   # 4 h-tiles
NC_ = C // P  # 8 c-tiles
FD = 512      # matmul free-dim chunk
SHIFT = 100.0
NEG_BIG = -1.0e6

f32 = mybir.dt.float32
f16 = mybir.dt.float16
bf16 = mybir.dt.bfloat16
u8 = mybir.dt.uint8
AF = mybir.ActivationFunctionType
ALU = mybir.AluOpType


def _emit(ctx: ExitStack, tc: "tile.TileContext", out_d, dec_d, ctx_d, mask_d, wa_d):
    nc = tc.nc

    pw = ctx.enter_context(tc.tile_pool(name="pw", bufs=1))
    pin = ctx.enter_context(tc.tile_pool(name="pin", bufs=2))
    pstage = ctx.enter_context(tc.tile_pool(name="pstage", bufs=2))
    ptr = ctx.enter_context(tc.tile_pool(name="ptr", bufs=2))
    ptmp = ctx.enter_context(tc.tile_pool(name="ptmp", bufs=2))
    pout = ctx.enter_context(tc.tile_pool(name="pout", bufs=2))
    pstat = ctx.enter_context(tc.tile_pool(name="pstat", bufs=2))
    ppsum = ctx.enter_context(
        tc.tile_pool(name="ppsum", bufs=2, space=bass.MemorySpace.PSUM)
    )

    def transpose(dst, src):
        # xbar transposes MUST all go through one queue (concurrent transposes
        # on two HWDGE queues corrupt data - verified on HW).
        nc.sync.dma_start(dst, src, transpose=True)

    # ---- once-per-core constants --------------------------------------------
    bias_tile = pw.tile([P, 1], f32, tag="bias")
    nc.gpsimd.memset(bias_tile[:], -SHIFT)
    ident = pw.tile([P, P], f32, tag="ident")
    make_identity(nc, ident[:])
    wz = pw.tile([P, FD], f16, tag="wz")
    nc.gpsimd.memset(wz[:], 0.0)

    # waT[p, ct, h] = W_a.T[ct*128+p, h]
    waT = pw.tile([P, NC_, H], f16, tag="waT")

    def warm_mm(ps):
        # dummy matmul: keeps the PE HAM activity monitor in the warm state
        # (transpose-mode matmuls don't count as PE-busy for HAM)
        nc.tensor.matmul(ps[:], wz[:, 0:P], wz[:], start=True, stop=True)

    def pe_transpose_block(dst_fp16, src_f32_tiles):
        """dst (fp16 [128, 4*128]) <- transposes of 4 f32 [128,128] tiles."""
        pst = ppsum.tile([P, FD], f32, tag="psh", bufs=2, name="pst")
        for i, srct in enumerate(src_f32_tiles):
            nc.tensor.transpose(pst[:, i * P : (i + 1) * P], srct, ident[:])
        nc.vector.tensor_copy(dst_fp16, pst[:])

    def load_wa():
        # gpsimd queue first slot (lands ~14us), f32 load -> PE transposes
        stg = pstage.tile([P, NH, C], f32, tag="wa_stg", bufs=1, name="wa_stg")
        nc.gpsimd.dma_start(stg[:], wa_d.rearrange("(a p) c -> p a c", p=P))
        for ct in range(NC_):
            pe_transpose_block(
                waT[:, ct, :],
                [stg[:, ht, ct * P : (ct + 1) * P] for ht in range(NH)],
            )

    # Per-batch tiles.
    ctxf = [None] * BLOC   # ctx fp16 natural  [p, st, c]
    ctxT = [None] * BLOC   # (lo, hi): half[p, ct, s_half] = ctx.T[ct*128+p, ...]
    decT = [None] * BLOC   # (lo, hi): half[p, kh, t_half] = dec.T[kh*128+p, ...]
    masku = [None] * BLOC  # mask u8 natural [p, tt, s]
    CI = [None] * BLOC     # CI fp16 [p, kh, s]
    attnT = [None] * BLOC  # 8 tiles [p, st, 128], one per t-tile

    ctx_stgs = {}

    def load_ctx_dma(b):
        """Two 2MB chunk loads on the gpsimd SWDGE queue."""
        cr = ctx_d[b].rearrange("(st p) c -> p st c", p=P)
        for half in range(2):
            sl = slice(half * 4, (half + 1) * 4)
            stg = pstage.tile([P, 4, C], f32, tag="ctx_stg", bufs=2,
                              name=f"ctx_stg{b}_{half}")
            nc.gpsimd.dma_start(stg[:], cr[:, sl, :])
            ctx_stgs[(b, half)] = stg

    def ctx_alloc(b):
        cf = pin.tile([P, NS, C], f16, tag="ctx_f16", bufs=2, name="ctx_f16")
        lo = ptr.tile([P, NC_, FD], f16, tag="ctxT_lo", bufs=1, name="ctxT_lo")
        hi = ptr.tile([P, NC_, FD], f16, tag="ctxT_hi", bufs=1, name="ctxT_hi")
        ctxf[b] = cf
        ctxT[b] = (lo, hi)

    def ctx_cast(b, half):
        """f32 staging -> fp16 natural layout (cf), on DVE."""
        stg = ctx_stgs[(b, half)]
        sl = slice(half * 4, (half + 1) * 4)
        nc.vector.tensor_copy(ctxf[b][:, sl, :], stg[:])

    def ctx_transpose_half_pe(b, half):
        """Startup path: PE transposes straight from f32 staging."""
        stg = ctx_stgs[(b, half)]
        dst = ctxT[b][half]
        for ct in range(NC_):
            pe_transpose_block(
                dst[:, ct, :],
                [stg[:, j, ct * P : (ct + 1) * P] for j in range(4)],
            )

    def ctx_transpose_xbar(b):
        """Steady path: 8 xbar transposes from the fp16 natural tile."""
        for j8 in range(NS):
            half, j = divmod(j8, 4)
            dst = ctxT[b][half]
            transpose(dst[:, :, j * P : (j + 1) * P], ctxf[b][:, j8, :])

    def load_dec_dma(b, eng):
        stg = pstage.tile([P, NT, H], f32, tag="dec_stg", bufs=1,
                          name=f"dec_stg{b}")
        eng.dma_start(stg[:], dec_d[b].rearrange("(tt p) h -> p tt h", p=P))
        return stg

    def dec_alloc(b):
        dlo = ptr.tile([P, NH, FD], f16, tag="decT_lo", bufs=1, name="decT_lo")
        dhi = ptr.tile([P, NH, FD], f16, tag="decT_hi", bufs=1, name="decT_hi")
        decT[b] = (dlo, dhi)

    def dec_transpose_pe(b, stg):
        dlo, dhi = decT[b]
        for half in range(2):
            dst = dlo if half == 0 else dhi
            for kh in range(NH):
                pe_transpose_block(
                    dst[:, kh, :],
                    [stg[:, half * 4 + tt, kh * P : (kh + 1) * P]
                     for tt in range(4)],
                )

    def dec_f16_alloc(b):
        return pin.tile([P, NT, H], f16, tag="dec_f16", bufs=1, name=f"dec_f16_{b}")

    def dec_cast(df, stg, half):
        sl = slice(half * 4, (half + 1) * 4)
        nc.vector.tensor_copy(df[:, sl, :], stg[:, sl, :])

    def dec_transpose_xbar(b, df):
        dlo, dhi = decT[b]
        for tt in range(NT):
            dst = dlo if tt < 4 else dhi
            transpose(dst[:, :, (tt % 4) * P : (tt % 4 + 1) * P], df[:, tt, :])

    def load_mask(b, eng):
        # shared buffer: b1's load WAR-waits on softmax(0)'s last mask read,
        # so it must go on the non-blocking gpsimd SWDGE queue
        mk = pin.tile([P, NT, S], u8, tag="mask", bufs=1, name=f"mask{b}")
        eng.dma_start(mk[:], mask_d[b].rearrange("(tt p) s -> p tt s", p=P))
        masku[b] = mk

    def mm1_begin(b):
        ci = ptr.tile([P, NH, S], f16, tag="CI", bufs=1, name="CI")
        CI[b] = ci

    def mm1_half(b, ns):
        ci = CI[b]
        rhs_t = ctxT[b][ns]
        for mh in range(NH):
            ps = ppsum.tile([P, FD], f32, tag="psh", bufs=2, name="psh")
            for ct in range(NC_):
                nc.tensor.matmul(
                    ps[:],
                    waT[:, ct, mh * P : (mh + 1) * P],
                    rhs_t[:, ct, :],
                    start=(ct == 0),
                    stop=(ct == NC_ - 1),
                )
            nc.vector.tensor_copy(ci[:, mh, ns * FD : (ns + 1) * FD], ps[:])

    def mm2_softmax(b, extra_slot=None):
        rs = pstat.tile([P, NT], f32, tag="rowsum")
        rr = pstat.tile([P, NT], f32, tag="rrec")
        aT = [
            ptr.tile([P, NS, P], f16, tag=f"attnT{mt}", bufs=1, name=f"attnT{mt}")
            for mt in range(NT)
        ]
        for mt in range(NT):
            ps = ppsum.tile([P, S], f32, tag="ps", bufs=3, name="ps")
            dth = decT[b][0] if mt < 4 else decT[b][1]
            for ns in range(2):
                for kh in range(NH):
                    nc.tensor.matmul(
                        ps[:, ns * FD : (ns + 1) * FD],
                        dth[:, kh, (mt % 4) * P : (mt % 4 + 1) * P],
                        CI[b][:, kh, ns * FD : (ns + 1) * FD],
                        start=(kh == 0),
                        stop=(kh == NH - 1),
                    )
            # masked = (mask * -1e6) + scores   (single DVE pass)
            sm = ptmp.tile([P, S], f32, tag="sm", bufs=2, name="sm")
            nc.vector.scalar_tensor_tensor(
                sm[:], masku[b][:, mt, :], NEG_BIG, ps[:], op0=ALU.mult, op1=ALU.add
            )
            # exp(masked - SHIFT) with fused rowsum
            ex = ptmp.tile([P, S], bf16, tag="ex", bufs=2, name="ex")
            nc.scalar.activation(
                ex[:], sm[:], AF.Exp, bias=bias_tile[:], scale=1.0,
                accum_out=rs[:, mt : mt + 1],
            )
            nc.vector.reciprocal(rr[:, mt : mt + 1], rs[:, mt : mt + 1])
            # attn = exp * (1/rowsum), fp16 in [0,1]; alternate DVE/ACT
            at = ptmp.tile([P, S], f16, tag="attn", bufs=3, name="attn")
            if mt % 2 == 0:
                nc.vector.tensor_scalar_mul(at[:], ex[:], rr[:, mt : mt + 1])
            else:
                nc.scalar.activation(
                    at[:], ex[:], AF.Copy, bias=0.0,
                    scale=rr[:, mt : mt + 1],
                )
            # attnT_mt[p, st, j] = attn[j, st*128+p]
            transpose(aT[mt][:], at[:])
            if extra_slot is not None:
                extra_slot(mt)
        attnT[b] = aT

    def mm3(b):
        for mt in range(NT):
            ps = ppsum.tile([P, S], f32, tag="ps", bufs=3, name="ps3")
            for nck in range(2):
                for ks in range(NS):
                    nc.tensor.matmul(
                        ps[:, nck * FD : (nck + 1) * FD],
                        attnT[b][mt][:, ks, :],
                        ctxf[b][:, ks, nck * FD : (nck + 1) * FD],
                        start=(ks == 0),
                        stop=(ks == NS - 1),
                    )
            ob = pout.tile([P, C], f16, tag="ob", bufs=2, name="ob")
            dst = out_d[b].rearrange("(tt p) c -> p tt c", p=P)[:, mt, :]
            if mt % 2 == 0:
                nc.scalar.copy(ob[:], ps[:])
                nc.scalar.dma_start(dst, ob[:])
            else:
                nc.vector.tensor_copy(ob[:], ps[:])
                nc.gpsimd.dma_start(dst, ob[:])

    # ---- schedule -----------------------------------------------------------
    wps = ppsum.tile([P, FD], f32, tag="psh", bufs=2, name="warm0")
    for _ in range(20):
        warm_mm(wps)

    # DMA issues. gpsimd SWDGE: wa, ctx b0, ctx b1, dec b1, mask b1
    # (+odd stores). scalar HWDGE: dec b0, mask b0 (ACT idle while blocked).
    load_wa()
    load_ctx_dma(0)
    load_ctx_dma(1)
    dec_stg0 = load_dec_dma(0, nc.scalar)
    load_mask(0, nc.scalar)
    dec_stg1 = load_dec_dma(1, nc.gpsimd)
    load_mask(1, nc.gpsimd)

    # batch 0 startup: waT/ctxT-b0 on PE (idle anyway); decT-b0 via xbar
    # (runs parallel to mm1 on the otherwise-free transpose queue)
    ctx_alloc(0)
    dec_alloc(0)
    df0 = dec_f16_alloc(0)
    mm1_begin(0)
    ctx_transpose_half_pe(0, 0)   # PE: ctxT-lo
    ctx_cast(0, 0)                # DVE; also unblocks ctx-b1-ch0 (WAR on stg)
    dec_cast(df0, dec_stg0, 0)
    dec_cast(df0, dec_stg0, 1)
    dec_transpose_xbar(0, df0)
    mm1_half(0, 0)                # PE: mm1 on lo while ch1 still landing
    ctx_transpose_half_pe(0, 1)   # PE: ctxT-hi
    ctx_cast(0, 1)
    mm1_half(0, 1)

    # batch 1 ctx prep (lands early thanks to the dedicated gpsimd stream)
    ctx_alloc(1)
    ctx_cast(1, 0)
    ctx_cast(1, 1)
    ctx_transpose_xbar(1)

    # dec b1 cast is chunked into softmax(0) DVE slots (its DMA lands late;
    # emitting it earlier would head-of-line block the softmax STT chain)
    dec_alloc(1)
    df1 = dec_f16_alloc(1)

    def b1_slot(mt):
        if mt in (4, 5):
            dec_cast(df1, dec_stg1, mt - 4)

    mm2_softmax(0, extra_slot=b1_slot)
    dec_transpose_xbar(1, df1)
    mm1_begin(1)
    mm1_half(1, 0)
    mm1_half(1, 1)
    mm3(0)
    mm2_softmax(1)
    mm3(1)


_BUILT = None


def _build():
    global _BUILT
    if _BUILT is not None:
        return _BUILT
    nc = bacc.Bacc("TRN2", target_bir_lowering=False, debug=False)
    dec_d = nc.dram_tensor("dec", [BLOC, T, H], f32, kind="ExternalInput")
    ctx_d = nc.dram_tensor("ctx", [BLOC, S, C], f32, kind="ExternalInput")
    mask_d = nc.dram_tensor("mask", [BLOC, T, S], u8, kind="ExternalInput")
    wa_d = nc.dram_tensor("wa", [H, C], f32, kind="ExternalInput")
    out_d = nc.dram_tensor("out", [BLOC, T, C], f16, kind="ExternalOutput")
    with tile.TileContext(nc) as tc, ExitStack() as ctx:
        _emit(ctx, tc, out_d.ap(), dec_d.ap(), ctx_d.ap(), mask_d.ap(), wa_d.ap())
    nc.compile()
    _BUILT = nc
    return nc


def make_in_maps(decoder_output, context, mask, W_a):
    decoder_output = np.ascontiguousarray(np.asarray(decoder_output, dtype=np.float32))
    context = np.ascontiguousarray(np.asarray(context, dtype=np.float32))
    mask_u8 = np.ascontiguousarray(np.asarray(mask)).astype(np.uint8)
    W_a = np.ascontiguousarray(np.asarray(W_a, dtype=np.float32))
    in_maps = []
    for i in range(N_CORES):
        sl = slice(i * BLOC, (i + 1) * BLOC)
        in_maps.append(
            {
                "dec": decoder_output[sl],
                "ctx": context[sl],
                "mask": mask_u8[sl],
                "wa": W_a,
            }
        )
    return in_maps


def kernel(decoder_output, context, mask, W_a, **run_kwargs):
    nc = _build()
    in_maps = make_in_maps(decoder_output, context, mask, W_a)
    res = run_bass_kernel_spmd(nc, in_maps, core_ids=list(range(N_CORES)), **run_kwargs)
    out = np.concatenate([res.results[i]["out"] for i in range(N_CORES)], axis=0)
    return out.astype(np.float32)


if __name__ == "__main__":
    nc = _build()
    print("build + compile OK")


# revision 13
# speedup vs baseline: 1.0057x; 1.0057x over previous
"""Trainium2 Bass kernel for masked attention (nn_Attention_77704548319841).

Reference computation per batch b:
    CI     = einsum('sc,hc->hs', context[b], W_a)          # [H, S]
    scores = einsum('th,hs->ts', dec[b], CI)               # [T, S]
    scores = where(mask, -1e6, scores)
    attn   = softmax(scores, axis=-1)
    out[b] = einsum('ts,sc->tc', attn, context[b])         # [T, 2H]

Sharding: pure data parallel over batch (16 batches / 8 cores = 2 per core).
W_a is replicated.

Per-core pipeline (matmuls on TensorE, fp16 operands, f32 PSUM accum):
  mm1: CI[h,s]     = W_aT.T @ ctxT        (lhsT = W_a.T  [C,H], rhs = ctx.T [C,S])
  mm2: scores[t,s] = decT.T @ CI          (lhsT = dec.T  [H,T], rhs = CI    [H,S])
  softmax (free-dim S):  masked = mask*-1e6 + scores   (one DVE scalar_tensor_tensor)
                         exp    = Exp(masked - SHIFT), rowsum via ACT accum_out
                         attn   = exp * (1/rowsum)     (fp16, values in [0,1])
  mm3: out[t,c]    = attnT.T @ ctx        (lhsT = attn.T [S,T], rhs = ctx [S,C])

PE order: startup transposes + mm1(0), softmax-mm(0), mm1(1), mm3(0),
softmax-mm(1), mm3(1).  This order makes the sync-queue transposes'
readiness order equal their need order (ctxT b1 -> attnT b0 -> decT b1 ->
attnT b1), which matters because the tile scheduler orders queue work by
readiness.

Measured platform facts this schedule is built around:
  - plain big DMA ~300GB/s per queue; HBM ~358GB/s per core aggregate
  - HWDGE queues (scalar/sync) BLOCK THE ISSUING ENGINE until ring credits
    free (~1 outstanding big DMA); the gpsimd SWDGE queue does not block
  - xbar transpose ~1.27us per call regardless of size; ALL transposes must
    stay on ONE queue (concurrent transposes from two queues corrupt data);
    explicit dep-chains between them add ~1.8us each - rely on natural
    queue serialization instead
  - Pool-engine (gpsimd) casts are ~6x slower than DVE casts - never use
  - PE p-state: long idle gaps are followed by a half-speed ramp window

Queue plan:
  gpsimd SWDGE : ctx b0 ch0/ch1, ctx b1 ch0/ch1, dec b1, mask b1 (the
                 urgent stream + WAR-gated mask; non-blocking issues),
                 then odd-mt output stores
  scalar queue : wa, dec b0, mask b0 (issued while ACT idle), even stores
  sync queue   : ALL xbar transposes: ctxT b1, attnT b0, decT b1, attnT b1
  TensorE      : matmuls + startup transposes of W_a/ctx-b0/dec-b0 (PE idle)
  VectorE      : f32->fp16 casts, psum evictions, masking, attn scale (half)
  ScalarE      : exp (+fused rowsum), attn scale (half), half mm3 evictions

Output is stored as fp16 (halves store traffic; ~1e-4 extra rel err) and
upcast to f32 on the host after the gather.

Softmax normalization is mathematically exact w.r.t. the reference: a constant
shift (instead of rowmax) leaves softmax unchanged; masked entries get
exp(s - 1e6 - SHIFT) == 0, identical to the reference's where(-1e6) after its
own exp underflow.
"""

import numpy as np
from contextlib import ExitStack

import concourse.bass as bass
import concourse.tile as tile
from concourse import bacc, mybir
from concourse.masks import make_identity
from concourse.bass_utils import run_bass_kernel_spmd

B, T, S, H = 16, 1024, 1024, 512
C = 2 * H
N_CORES = 8
BLOC = B // N_CORES  # batches per core
P = 128
NT = T // P   # 8 t-tiles
NS = S // P   # 8 s-tiles
NH = H // P   # 4 h-tiles
NC_ = C // P  # 8 c-tiles
FD = 512      # matmul free-dim chunk
SHIFT = 100.0
NEG_BIG = -1.0e6

f32 = mybir.dt.float32
f16 = mybir.dt.float16
bf16 = mybir.dt.bfloat16
u8 = mybir.dt.uint8
AF = mybir.ActivationFunctionType
ALU = mybir.AluOpType


def _emit(ctx: ExitStack, tc: "tile.TileContext", out_d, dec_d, ctx_d, mask_d, wa_d):
    nc = tc.nc

    pw = ctx.enter_context(tc.tile_pool(name="pw", bufs=1))
    pin = ctx.enter_context(tc.tile_pool(name="pin", bufs=2))
    pstage = ctx.enter_context(tc.tile_pool(name="pstage", bufs=2))
    ptr = ctx.enter_context(tc.tile_pool(name="ptr", bufs=2))
    ptmp = ctx.enter_context(tc.tile_pool(name="ptmp", bufs=2))
    pout = ctx.enter_context(tc.tile_pool(name="pout", bufs=2))
    pstat = ctx.enter_context(tc.tile_pool(name="pstat", bufs=2))
    ppsum = ctx.enter_context(
        tc.tile_pool(name="ppsum", bufs=2, space=bass.MemorySpace.PSUM)
    )

    def transpose(dst, src):
        # xbar transposes MUST all go through one queue (concurrent transposes
        # on two HWDGE queues corrupt data - verified on HW).
        nc.sync.dma_start(dst, src, transpose=True)

    # ---- once-per-core constants --------------------------------------------
    bias_tile = pw.tile([P, 1], f32, tag="bias")
    nc.gpsimd.memset(bias_tile[:], -SHIFT)
    ident = pw.tile([P, P], f32, tag="ident")
    make_identity(nc, ident[:])
    wz = pw.tile([P, FD], f16, tag="wz")
    nc.gpsimd.memset(wz[:], 0.0)

    # waT[p, ct, h] = W_a.T[ct*128+p, h]
    waT = pw.tile([P, NC_, H], f16, tag="waT")

    def warm_mm(ps):
        # dummy matmul: keeps the PE HAM activity monitor in the warm state
        # (transpose-mode matmuls don't count as PE-busy for HAM)
        nc.tensor.matmul(ps[:], wz[:, 0:P], wz[:], start=True, stop=True)

    def pe_transpose_block(dst_fp16, src_f32_tiles):
        """dst (fp16 [128, 4*128]) <- transposes of 4 f32 [128,128] tiles."""
        pst = ppsum.tile([P, FD], f32, tag="psh", bufs=2, name="pst")
        for i, srct in enumerate(src_f32_tiles):
            nc.tensor.transpose(pst[:, i * P : (i + 1) * P], srct, ident[:])
        nc.vector.tensor_copy(dst_fp16, pst[:])

    def load_wa():
        # scalar queue first slot (ACT idle while ring-blocked)
        stg = pstage.tile([P, NH, C], f32, tag="wa_stg", bufs=1, name="wa_stg")
        nc.scalar.dma_start(stg[:], wa_d.rearrange("(a p) c -> p a c", p=P))
        return stg

    def wa_transpose_pe(stg):
        for ct in range(NC_):
            pe_transpose_block(
                waT[:, ct, :],
                [stg[:, ht, ct * P : (ct + 1) * P] for ht in range(NH)],
            )

    # Per-batch tiles.
    ctxf = [None] * BLOC   # ctx fp16 natural  [p, st, c]
    ctxT = [None] * BLOC   # (lo, hi): half[p, ct, s_half] = ctx.T[ct*128+p, ...]
    decT = [None] * BLOC   # (lo, hi): half[p, kh, t_half] = dec.T[kh*128+p, ...]
    masku = [None] * BLOC  # mask u8 natural [p, tt, s]
    CI = [None] * BLOC     # CI fp16 [p, kh, s]
    attnT = [None] * BLOC  # 8 tiles [p, st, 128], one per t-tile

    ctx_stgs = {}

    def load_ctx_dma(b):
        """Two 2MB chunk loads on the gpsimd SWDGE queue."""
        cr = ctx_d[b].rearrange("(st p) c -> p st c", p=P)
        for half in range(2):
            sl = slice(half * 4, (half + 1) * 4)
            stg = pstage.tile([P, 4, C], f32, tag="ctx_stg", bufs=2,
                              name=f"ctx_stg{b}_{half}")
            nc.gpsimd.dma_start(stg[:], cr[:, sl, :])
            ctx_stgs[(b, half)] = stg

    def ctx_alloc(b):
        cf = pin.tile([P, NS, C], f16, tag="ctx_f16", bufs=2, name="ctx_f16")
        lo = ptr.tile([P, NC_, FD], f16, tag="ctxT_lo", bufs=1, name="ctxT_lo")
        hi = ptr.tile([P, NC_, FD], f16, tag="ctxT_hi", bufs=1, name="ctxT_hi")
        ctxf[b] = cf
        ctxT[b] = (lo, hi)

    def ctx_cast(b, half):
        """f32 staging -> fp16 natural layout (cf), on DVE."""
        stg = ctx_stgs[(b, half)]
        sl = slice(half * 4, (half + 1) * 4)
        nc.vector.tensor_copy(ctxf[b][:, sl, :], stg[:])

    def ctx_transpose_half_pe(b, half):
        """Startup path: PE transposes straight from f32 staging."""
        stg = ctx_stgs[(b, half)]
        dst = ctxT[b][half]
        for ct in range(NC_):
            pe_transpose_block(
                dst[:, ct, :],
                [stg[:, j, ct * P : (ct + 1) * P] for j in range(4)],
            )

    def ctx_transpose_xbar(b):
        """Steady path: 8 xbar transposes from the fp16 natural tile."""
        for j8 in range(NS):
            half, j = divmod(j8, 4)
            dst = ctxT[b][half]
            transpose(dst[:, :, j * P : (j + 1) * P], ctxf[b][:, j8, :])

    def load_dec_dma(b, eng):
        stg = pstage.tile([P, NT, H], f32, tag="dec_stg", bufs=1,
                          name=f"dec_stg{b}")
        eng.dma_start(stg[:], dec_d[b].rearrange("(tt p) h -> p tt h", p=P))
        return stg

    def dec_alloc(b):
        dlo = ptr.tile([P, NH, FD], f16, tag="decT_lo", bufs=1, name="decT_lo")
        dhi = ptr.tile([P, NH, FD], f16, tag="decT_hi", bufs=1, name="decT_hi")
        decT[b] = (dlo, dhi)

    def dec_transpose_pe(b, stg):
        dlo, dhi = decT[b]
        for half in range(2):
            dst = dlo if half == 0 else dhi
            for kh in range(NH):
                pe_transpose_block(
                    dst[:, kh, :],
                    [stg[:, half * 4 + tt, kh * P : (kh + 1) * P]
                     for tt in range(4)],
                )

    def dec_f16_alloc(b):
        return pin.tile([P, NT, H], f16, tag="dec_f16", bufs=1, name=f"dec_f16_{b}")

    def dec_cast(df, stg, half, eng):
        sl = slice(half * 4, (half + 1) * 4)
        if eng is nc.scalar:
            nc.scalar.copy(df[:, sl, :], stg[:, sl, :])
        else:
            eng.tensor_copy(df[:, sl, :], stg[:, sl, :])

    def dec_transpose_xbar(b, df):
        dlo, dhi = decT[b]
        for tt in range(NT):
            dst = dlo if tt < 4 else dhi
            transpose(dst[:, :, (tt % 4) * P : (tt % 4 + 1) * P], df[:, tt, :])

    def load_mask(b, eng):
        # shared buffer: b1's load WAR-waits on softmax(0)'s last mask read,
        # so it must go on the non-blocking gpsimd SWDGE queue
        mk = pin.tile([P, NT, S], u8, tag="mask", bufs=1, name=f"mask{b}")
        eng.dma_start(mk[:], mask_d[b].rearrange("(tt p) s -> p tt s", p=P))
        masku[b] = mk

    def mm1_begin(b):
        ci = ptr.tile([P, NH, S], f16, tag="CI", bufs=1, name="CI")
        CI[b] = ci

    def mm1_half(b, ns):
        ci = CI[b]
        rhs_t = ctxT[b][ns]
        for mh in range(NH):
            ps = ppsum.tile([P, FD], f32, tag="psh", bufs=2, name="psh")
            for ct in range(NC_):
                nc.tensor.matmul(
                    ps[:],
                    waT[:, ct, mh * P : (mh + 1) * P],
                    rhs_t[:, ct, :],
                    start=(ct == 0),
                    stop=(ct == NC_ - 1),
                )
            nc.vector.tensor_copy(ci[:, mh, ns * FD : (ns + 1) * FD], ps[:])

    def mm2_softmax(b, extra_slot=None):
        rs = pstat.tile([P, NT], f32, tag="rowsum")
        rr = pstat.tile([P, NT], f32, tag="rrec")
        aT = [
            ptr.tile([P, NS, P], f16, tag=f"attnT{mt}", bufs=1, name=f"attnT{mt}")
            for mt in range(NT)
        ]
        for mt in range(NT):
            ps = ppsum.tile([P, S], f32, tag="ps", bufs=3, name="ps")
            dth = decT[b][0] if mt < 4 else decT[b][1]
            for ns in range(2):
                for kh in range(NH):
                    nc.tensor.matmul(
                        ps[:, ns * FD : (ns + 1) * FD],
                        dth[:, kh, (mt % 4) * P : (mt % 4 + 1) * P],
                        CI[b][:, kh, ns * FD : (ns + 1) * FD],
                        start=(kh == 0),
                        stop=(kh == NH - 1),
                    )
            # masked = (mask * -1e6) + scores   (single DVE pass)
            sm = ptmp.tile([P, S], f32, tag="sm", bufs=2, name="sm")
            nc.vector.scalar_tensor_tensor(
                sm[:], masku[b][:, mt, :], NEG_BIG, ps[:], op0=ALU.mult, op1=ALU.add
            )
            # exp(masked - SHIFT) with fused rowsum
            ex = ptmp.tile([P, S], bf16, tag="ex", bufs=2, name="ex")
            nc.scalar.activation(
                ex[:], sm[:], AF.Exp, bias=bias_tile[:], scale=1.0,
                accum_out=rs[:, mt : mt + 1],
            )
            nc.vector.reciprocal(rr[:, mt : mt + 1], rs[:, mt : mt + 1])
            # attn = exp * (1/rowsum), fp16 in [0,1]; alternate DVE/ACT
            at = ptmp.tile([P, S], f16, tag="attn", bufs=3, name="attn")
            if mt % 2 == 0:
                nc.vector.tensor_scalar_mul(at[:], ex[:], rr[:, mt : mt + 1])
            else:
                nc.scalar.activation(
                    at[:], ex[:], AF.Copy, bias=0.0,
                    scale=rr[:, mt : mt + 1],
                )
            # attnT_mt[p, st, j] = attn[j, st*128+p]
            transpose(aT[mt][:], at[:])
            if extra_slot is not None:
                extra_slot(mt)
        attnT[b] = aT

    def mm3(b):
        for mt in range(NT):
            ps = ppsum.tile([P, S], f32, tag="ps", bufs=3, name="ps3")
            for nck in range(2):
                for ks in range(NS):
                    nc.tensor.matmul(
                        ps[:, nck * FD : (nck + 1) * FD],
                        attnT[b][mt][:, ks, :],
                        ctxf[b][:, ks, nck * FD : (nck + 1) * FD],
                        start=(ks == 0),
                        stop=(ks == NS - 1),
                    )
            ob = pout.tile([P, C], f16, tag="ob", bufs=2, name="ob")
            dst = out_d[b].rearrange("(tt p) c -> p tt c", p=P)[:, mt, :]
            if mt % 2 == 0:
                nc.scalar.copy(ob[:], ps[:])
                nc.scalar.dma_start(dst, ob[:])
            else:
                nc.vector.tensor_copy(ob[:], ps[:])
                nc.gpsimd.dma_start(dst, ob[:])

    # ---- schedule -----------------------------------------------------------
    wps = ppsum.tile([P, FD], f32, tag="psh", bufs=2, name="warm0")
    for _ in range(32):
        warm_mm(wps)

    # DMA issues. scalar HWDGE: wa, dec b0, mask b0 (ACT idle while
    # ring-blocked). gpsimd SWDGE: ctx b0, ctx b1, dec b1, mask b1 (+odd
    # stores).
    wa_stg = load_wa()
    load_ctx_dma(0)
    load_ctx_dma(1)
    dec_stg0 = load_dec_dma(0, nc.scalar)
    load_mask(0, nc.scalar)
    dec_stg1 = load_dec_dma(1, nc.gpsimd)
    load_mask(1, nc.gpsimd)

    # batch 0 startup: ctxT-lo (ready first) then waT on PE; decT-b0 via
    # xbar with its cast on the idle ACT engine (keeps DVE for the
    # mm1-critical psum evictions)
    ctx_alloc(0)
    dec_alloc(0)
    df0 = dec_f16_alloc(0)
    mm1_begin(0)
    ctx_transpose_half_pe(0, 0)   # PE: ctxT-lo (ctx ch0 lands before wa)
    ctx_cast(0, 0)                # DVE; also unblocks ctx-b1-ch0 (WAR on stg)
    wa_transpose_pe(wa_stg)       # PE: waT
    dec_cast(df0, dec_stg0, 0, nc.scalar)
    dec_cast(df0, dec_stg0, 1, nc.scalar)
    dec_transpose_xbar(0, df0)
    mm1_half(0, 0)                # PE: mm1 on lo while ch1 still landing
    ctx_transpose_half_pe(0, 1)   # PE: ctxT-hi
    ctx_cast(0, 1)
    mm1_half(0, 1)

    # batch 1 ctx prep (lands early thanks to the dedicated gpsimd stream)
    ctx_alloc(1)
    ctx_cast(1, 0)
    ctx_cast(1, 1)
    ctx_transpose_xbar(1)

    # dec b1 cast is chunked into softmax(0) DVE slots (its DMA lands late;
    # emitting it earlier would head-of-line block the softmax STT chain)
    dec_alloc(1)
    df1 = dec_f16_alloc(1)

    def b1_slot(mt):
        if mt in (5, 6):
            dec_cast(df1, dec_stg1, mt - 5, nc.vector)

    mm2_softmax(0, extra_slot=b1_slot)
    dec_transpose_xbar(1, df1)
    mm1_begin(1)
    mm1_half(1, 0)
    mm1_half(1, 1)
    mm3(0)
    mm2_softmax(1)
    mm3(1)


_BUILT = None


def _build():
    global _BUILT
    if _BUILT is not None:
        return _BUILT
    nc = bacc.Bacc("TRN2", target_bir_lowering=False, debug=False)
    dec_d = nc.dram_tensor("dec", [BLOC, T, H], f32, kind="ExternalInput")
    ctx_d = nc.dram_tensor("ctx", [BLOC, S, C], f32, kind="ExternalInput")
    mask_d = nc.dram_tensor("mask", [BLOC, T, S], u8, kind="ExternalInput")
    wa_d = nc.dram_tensor("wa", [H, C], f32, kind="ExternalInput")
    out_d = nc.dram_tensor("out", [BLOC, T, C], f16, kind="ExternalOutput")
    with tile.TileContext(nc) as tc, ExitStack() as ctx:
        _emit(ctx, tc, out_d.ap(), dec_d.ap(), ctx_d.ap(), mask_d.ap(), wa_d.ap())
    nc.compile()
    _BUILT = nc
    return nc


def make_in_maps(decoder_output, context, mask, W_a):
    decoder_output = np.ascontiguousarray(np.asarray(decoder_output, dtype=np.float32))
    context = np.ascontiguousarray(np.asarray(context, dtype=np.float32))
    mask_u8 = np.ascontiguousarray(np.asarray(mask)).astype(np.uint8)
    W_a = np.ascontiguousarray(np.asarray(W_a, dtype=np.float32))
    in_maps = []
    for i in range(N_CORES):
        sl = slice(i * BLOC, (i + 1) * BLOC)
        in_maps.append(
            {
                "dec": decoder_output[sl],
                "ctx": context[sl],
                "mask": mask_u8[sl],
                "wa": W_a,
            }
        )
    return in_maps


def kernel(decoder_output, context, mask, W_a, **run_kwargs):
    nc = _build()
    in_maps = make_in_maps(decoder_output, context, mask, W_a)
    res = run_bass_kernel_spmd(nc, in_maps, core_ids=list(range(N_CORES)), **run_kwargs)
    out = np.concatenate([res.results[i]["out"] for i in range(N_CORES)], axis=0)
    return out.astype(np.float32)


if __name__ == "__main__":
    nc = _build()
    print("build + compile OK")


# revision 14
# speedup vs baseline: 1.0914x; 1.0852x over previous
"""Trainium2 Bass kernel for masked attention (nn_Attention_77704548319841).

Reference computation per batch b:
    CI     = einsum('sc,hc->hs', context[b], W_a)          # [H, S]
    scores = einsum('th,hs->ts', dec[b], CI)               # [T, S]
    scores = where(mask, -1e6, scores)
    attn   = softmax(scores, axis=-1)
    out[b] = einsum('ts,sc->tc', attn, context[b])         # [T, 2H]

Sharding: pure data parallel over batch (16 batches / 8 cores = 2 per core).
W_a is replicated.

Per-core pipeline (matmuls on TensorE, fp16 operands, f32 PSUM accum):
  mm1: CI[h,s]     = W_aT.T @ ctxT        (lhsT = W_a.T  [C,H], rhs = ctx.T [C,S])
  mm2: scores[t,s] = decT.T @ CI          (lhsT = dec.T  [H,T], rhs = CI    [H,S])
  softmax (free-dim S):  masked = mask*-1e6 + scores   (one DVE scalar_tensor_tensor)
                         exp    = Exp(masked - SHIFT), rowsum via ACT accum_out
                         attn   = exp * (1/rowsum)     (fp16, values in [0,1])
  mm3: out[t,c]    = attnT.T @ ctx        (lhsT = attn.T [S,T], rhs = ctx [S,C])

PE order: startup transposes + mm1(0), softmax-mm(0), mm1(1), mm3(0),
softmax-mm(1), mm3(1).  This order makes the sync-queue transposes'
readiness order equal their need order (ctxT b1 -> attnT b0 -> decT b1 ->
attnT b1), which matters because the tile scheduler orders queue work by
readiness.

Measured platform facts this schedule is built around:
  - plain big DMA ~300GB/s per queue; HBM ~358GB/s per core aggregate
  - HWDGE queues (scalar/sync) BLOCK THE ISSUING ENGINE until ring credits
    free (~1 outstanding big DMA); the gpsimd SWDGE queue does not block
  - xbar transpose ~1.27us per call regardless of size; ALL transposes must
    stay on ONE queue (concurrent transposes from two queues corrupt data);
    explicit dep-chains between them add ~1.8us each - rely on natural
    queue serialization instead
  - Pool-engine (gpsimd) casts are ~6x slower than DVE casts - never use
  - PE p-state: long idle gaps are followed by a half-speed ramp window

Queue plan:
  gpsimd SWDGE : ctx b0 ch0/ch1, ctx b1 ch0/ch1, dec b1, mask b1 (the
                 urgent stream + WAR-gated mask; non-blocking issues),
                 then odd-mt output stores
  scalar queue : wa, dec b0, mask b0 (issued while ACT idle), even stores
  sync queue   : ALL xbar transposes: ctxT b1, attnT b0, decT b1, attnT b1
  TensorE      : matmuls + startup transposes of W_a/ctx-b0/dec-b0 (PE idle)
  VectorE      : f32->fp16 casts, psum evictions, masking, attn scale (half)
  ScalarE      : exp (+fused rowsum), attn scale (half), half mm3 evictions

Output is stored as fp16 (halves store traffic; ~1e-4 extra rel err) and
upcast to f32 on the host after the gather.

Softmax normalization is mathematically exact w.r.t. the reference: a constant
shift (instead of rowmax) leaves softmax unchanged; masked entries get
exp(s - 1e6 - SHIFT) == 0, identical to the reference's where(-1e6) after its
own exp underflow.
"""

import numpy as np
from contextlib import ExitStack

import concourse.bass as bass
import concourse.tile as tile
from concourse import bacc, mybir
from concourse.masks import make_identity
from concourse.bass_utils import run_bass_kernel_spmd

B, T, S, H = 16, 1024, 1024, 512
C = 2 * H
N_CORES = 8
BLOC = B // N_CORES  # batches per core
P = 128
NT = T // P   # 8 t-tiles
NS = S // P   # 8 s-tiles
NH = H // P   # 4 h-tiles
NC_ = C // P  # 8 c-tiles
FD = 512      # matmul free-dim chunk
SHIFT = 100.0
NEG_BIG = -1.0e6

f32 = mybir.dt.float32
f16 = mybir.dt.float16
bf16 = mybir.dt.bfloat16
u8 = mybir.dt.uint8
AF = mybir.ActivationFunctionType
ALU = mybir.AluOpType


def _emit(ctx: ExitStack, tc: "tile.TileContext", out_d, dec_d, ctx_d, mask_d, wa_d):
    nc = tc.nc

    pw = ctx.enter_context(tc.tile_pool(name="pw", bufs=1))
    pin = ctx.enter_context(tc.tile_pool(name="pin", bufs=2))
    pstage = ctx.enter_context(tc.tile_pool(name="pstage", bufs=2))
    ptr = ctx.enter_context(tc.tile_pool(name="ptr", bufs=2))
    ptmp = ctx.enter_context(tc.tile_pool(name="ptmp", bufs=2))
    pout = ctx.enter_context(tc.tile_pool(name="pout", bufs=2))
    pstat = ctx.enter_context(tc.tile_pool(name="pstat", bufs=2))
    ppsum = ctx.enter_context(
        tc.tile_pool(name="ppsum", bufs=2, space=bass.MemorySpace.PSUM)
    )

    def transpose(dst, src):
        # xbar transposes MUST all go through one queue (concurrent transposes
        # on two HWDGE queues corrupt data - verified on HW).
        nc.sync.dma_start(dst, src, transpose=True)

    # ---- once-per-core constants --------------------------------------------
    bias_tile = pw.tile([P, 1], f32, tag="bias")
    nc.gpsimd.memset(bias_tile[:], -SHIFT)
    ident = pw.tile([P, P], f32, tag="ident")
    make_identity(nc, ident[:])
    wz = pw.tile([P, FD], f16, tag="wz")
    nc.gpsimd.memset(wz[:], 0.0)

    # waT[p, ct, h] = W_a.T[ct*128+p, h]
    waT = pw.tile([P, NC_, H], f16, tag="waT")

    def warm_mm(ps):
        # dummy matmul: keeps the PE HAM activity monitor in the warm state
        # (transpose-mode matmuls don't count as PE-busy for HAM)
        nc.tensor.matmul(ps[:], wz[:, 0:P], wz[:], start=True, stop=True)

    def pe_transpose_block(dst_fp16, src_f32_tiles):
        """dst (fp16 [128, 4*128]) <- transposes of 4 f32 [128,128] tiles."""
        pst = ppsum.tile([P, FD], f32, tag="psh", bufs=2, name="pst")
        for i, srct in enumerate(src_f32_tiles):
            nc.tensor.transpose(pst[:, i * P : (i + 1) * P], srct, ident[:])
        nc.vector.tensor_copy(dst_fp16, pst[:])

    def load_wa():
        # scalar queue first slot (ACT idle while ring-blocked)
        stg = pstage.tile([P, NH, C], f32, tag="wa_stg", bufs=1, name="wa_stg")
        nc.scalar.dma_start(stg[:], wa_d.rearrange("(a p) c -> p a c", p=P))
        for ct in range(NC_):
            pe_transpose_block(
                waT[:, ct, :],
                [stg[:, ht, ct * P : (ct + 1) * P] for ht in range(NH)],
            )

    # Per-batch tiles.
    ctxf = [None] * BLOC   # ctx fp16 natural  [p, st, c]
    ctxT = [None] * BLOC   # (lo, hi): half[p, ct, s_half] = ctx.T[ct*128+p, ...]
    decT = [None] * BLOC   # (lo, hi): half[p, kh, t_half] = dec.T[kh*128+p, ...]
    masku = [None] * BLOC  # mask u8 natural [p, tt, s]
    CI = [None] * BLOC     # CI fp16 [p, kh, s]
    attnT = [None] * BLOC  # 8 tiles [p, st, 128], one per t-tile

    ctx_stgs = {}

    def load_ctx_dma(b):
        """Two 2MB chunk loads on the gpsimd SWDGE queue."""
        cr = ctx_d[b].rearrange("(st p) c -> p st c", p=P)
        for half in range(2):
            sl = slice(half * 4, (half + 1) * 4)
            stg = pstage.tile([P, 4, C], f32, tag="ctx_stg", bufs=2,
                              name=f"ctx_stg{b}_{half}")
            nc.gpsimd.dma_start(stg[:], cr[:, sl, :])
            ctx_stgs[(b, half)] = stg

    def ctx_alloc(b):
        cf = pin.tile([P, NS, C], f16, tag="ctx_f16", bufs=2, name="ctx_f16")
        lo = ptr.tile([P, NC_, FD], f16, tag="ctxT_lo", bufs=1, name="ctxT_lo")
        hi = ptr.tile([P, NC_, FD], f16, tag="ctxT_hi", bufs=1, name="ctxT_hi")
        ctxf[b] = cf
        ctxT[b] = (lo, hi)

    def ctx_cast(b, half):
        """f32 staging -> fp16 natural layout (cf), on DVE."""
        stg = ctx_stgs[(b, half)]
        sl = slice(half * 4, (half + 1) * 4)
        nc.vector.tensor_copy(ctxf[b][:, sl, :], stg[:])

    def ctx_transpose_half_pe(b, half):
        """Startup path: PE transposes straight from f32 staging."""
        stg = ctx_stgs[(b, half)]
        dst = ctxT[b][half]
        for ct in range(NC_):
            pe_transpose_block(
                dst[:, ct, :],
                [stg[:, j, ct * P : (ct + 1) * P] for j in range(4)],
            )

    def ctx_transpose_xbar(b):
        """Steady path: 8 xbar transposes from the fp16 natural tile."""
        for j8 in range(NS):
            half, j = divmod(j8, 4)
            dst = ctxT[b][half]
            transpose(dst[:, :, j * P : (j + 1) * P], ctxf[b][:, j8, :])

    def load_dec_dma(b, eng):
        stg = pstage.tile([P, NT, H], f32, tag=f"dec_stg{b}", bufs=1,
                          name=f"dec_stg{b}")
        eng.dma_start(stg[:], dec_d[b].rearrange("(tt p) h -> p tt h", p=P))
        return stg

    def dec_alloc(b):
        dlo = ptr.tile([P, NH, FD], f16, tag="decT_lo", bufs=1, name="decT_lo")
        dhi = ptr.tile([P, NH, FD], f16, tag="decT_hi", bufs=1, name="decT_hi")
        decT[b] = (dlo, dhi)

    def dec_transpose_pe(b, stg):
        dlo, dhi = decT[b]
        for half in range(2):
            dst = dlo if half == 0 else dhi
            for kh in range(NH):
                pe_transpose_block(
                    dst[:, kh, :],
                    [stg[:, half * 4 + tt, kh * P : (kh + 1) * P]
                     for tt in range(4)],
                )

    def dec_transpose_pe(b, stg):
        dlo, dhi = decT[b]
        for half in range(2):
            dst = dlo if half == 0 else dhi
            for kh in range(NH):
                pe_transpose_block(
                    dst[:, kh, :],
                    [stg[:, half * 4 + tt, kh * P : (kh + 1) * P]
                     for tt in range(4)],
                )

    def dec_cast_transpose_xbar(b, stg):
        df = pin.tile([P, NT, H], f16, tag="dec_f16", bufs=1, name="dec_f16")
        nc.vector.tensor_copy(df[:], stg[:])
        dlo, dhi = decT[b]
        for tt in range(NT):
            dst = dlo if tt < 4 else dhi
            transpose(dst[:, :, (tt % 4) * P : (tt % 4 + 1) * P], df[:, tt, :])

    def load_mask(b, eng):
        # shared buffer: b1's load WAR-waits on softmax(0)'s last mask read,
        # so it must go on the non-blocking gpsimd SWDGE queue
        mk = pin.tile([P, NT, S], u8, tag="mask", bufs=1, name=f"mask{b}")
        eng.dma_start(mk[:], mask_d[b].rearrange("(tt p) s -> p tt s", p=P))
        masku[b] = mk

    def mm1_begin(b):
        ci = ptr.tile([P, NH, S], f16, tag="CI", bufs=1, name="CI")
        CI[b] = ci

    def mm1_half(b, ns):
        ci = CI[b]
        rhs_t = ctxT[b][ns]
        for mh in range(NH):
            ps = ppsum.tile([P, FD], f32, tag="psh", bufs=2, name="psh")
            for ct in range(NC_):
                nc.tensor.matmul(
                    ps[:],
                    waT[:, ct, mh * P : (mh + 1) * P],
                    rhs_t[:, ct, :],
                    start=(ct == 0),
                    stop=(ct == NC_ - 1),
                )
            nc.vector.tensor_copy(ci[:, mh, ns * FD : (ns + 1) * FD], ps[:])

    def mm2_softmax(b, extra_slot=None):
        rs = pstat.tile([P, NT], f32, tag="rowsum")
        rr = pstat.tile([P, NT], f32, tag="rrec")
        aT = [
            ptr.tile([P, NS, P], f16, tag=f"attnT{mt}", bufs=1, name=f"attnT{mt}")
            for mt in range(NT)
        ]
        for mt in range(NT):
            ps = ppsum.tile([P, S], f32, tag="ps", bufs=3, name="ps")
            dth = decT[b][0] if mt < 4 else decT[b][1]
            for ns in range(2):
                for kh in range(NH):
                    nc.tensor.matmul(
                        ps[:, ns * FD : (ns + 1) * FD],
                        dth[:, kh, (mt % 4) * P : (mt % 4 + 1) * P],
                        CI[b][:, kh, ns * FD : (ns + 1) * FD],
                        start=(kh == 0),
                        stop=(kh == NH - 1),
                    )
            # masked = (mask * -1e6) + scores   (single DVE pass)
            sm = ptmp.tile([P, S], f32, tag="sm", bufs=2, name="sm")
            nc.vector.scalar_tensor_tensor(
                sm[:], masku[b][:, mt, :], NEG_BIG, ps[:], op0=ALU.mult, op1=ALU.add
            )
            # exp(masked - SHIFT) with fused rowsum
            ex = ptmp.tile([P, S], bf16, tag="ex", bufs=2, name="ex")
            nc.scalar.activation(
                ex[:], sm[:], AF.Exp, bias=bias_tile[:], scale=1.0,
                accum_out=rs[:, mt : mt + 1],
            )
            nc.vector.reciprocal(rr[:, mt : mt + 1], rs[:, mt : mt + 1])
            # attn = exp * (1/rowsum), fp16 in [0,1]; alternate DVE/ACT
            at = ptmp.tile([P, S], f16, tag="attn", bufs=3, name="attn")
            if mt % 2 == 0:
                nc.vector.tensor_scalar_mul(at[:], ex[:], rr[:, mt : mt + 1])
            else:
                nc.scalar.activation(
                    at[:], ex[:], AF.Copy, bias=0.0,
                    scale=rr[:, mt : mt + 1],
                )
            # attnT_mt[p, st, j] = attn[j, st*128+p]
            transpose(aT[mt][:], at[:])
            if extra_slot is not None:
                extra_slot(mt)
        attnT[b] = aT

    def mm3(b):
        for mt in range(NT):
            ps = ppsum.tile([P, S], f32, tag="ps", bufs=3, name="ps3")
            for nck in range(2):
                for ks in range(NS):
                    nc.tensor.matmul(
                        ps[:, nck * FD : (nck + 1) * FD],
                        attnT[b][mt][:, ks, :],
                        ctxf[b][:, ks, nck * FD : (nck + 1) * FD],
                        start=(ks == 0),
                        stop=(ks == NS - 1),
                    )
            ob = pout.tile([P, C], f16, tag="ob", bufs=2, name="ob")
            dst = out_d[b].rearrange("(tt p) c -> p tt c", p=P)[:, mt, :]
            if mt % 2 == 0:
                nc.scalar.copy(ob[:], ps[:])
                nc.scalar.dma_start(dst, ob[:])
            else:
                nc.vector.tensor_copy(ob[:], ps[:])
                nc.gpsimd.dma_start(dst, ob[:])

    # ---- schedule -----------------------------------------------------------
    wps = ppsum.tile([P, FD], f32, tag="psh", bufs=2, name="warm0")
    for _ in range(28):
        warm_mm(wps)

    # DMA issues. gpsimd SWDGE: ctx b0, ctx b1, dec b1, mask b1 (+odd
    # stores). scalar HWDGE: wa, dec b0, mask b0 (ACT idle while blocked).
    load_wa()
    load_ctx_dma(0)
    load_ctx_dma(1)
    dec_stg0 = load_dec_dma(0, nc.scalar)
    load_mask(0, nc.scalar)
    dec_stg1 = load_dec_dma(1, nc.gpsimd)
    load_mask(1, nc.gpsimd)

    # batch 0 startup: PE transposes from f32 staging, mm1 slotted between
    ctx_alloc(0)
    dec_alloc(0)
    mm1_begin(0)
    ctx_transpose_half_pe(0, 0)   # PE: ctxT-lo
    ctx_cast(0, 0)                # DVE; also unblocks ctx-b1-ch0 (WAR on stg)
    mm1_half(0, 0)                # PE: mm1 on lo while ch1 still landing
    ctx_transpose_half_pe(0, 1)   # PE: ctxT-hi
    ctx_cast(0, 1)
    mm1_half(0, 1)
    dec_transpose_pe(0, dec_stg0)

    # batch 1 ctx prep (lands early thanks to the dedicated gpsimd stream)
    ctx_alloc(1)
    ctx_cast(1, 0)
    ctx_cast(1, 1)
    ctx_transpose_xbar(1)

    mm2_softmax(0)
    dec_alloc(1)
    dec_cast_transpose_xbar(1, dec_stg1)
    mm1_begin(1)
    mm1_half(1, 0)
    mm1_half(1, 1)
    mm3(0)
    mm2_softmax(1)
    mm3(1)


_BUILT = None


def _build():
    global _BUILT
    if _BUILT is not None:
        return _BUILT
    nc = bacc.Bacc("TRN2", target_bir_lowering=False, debug=False)
    dec_d = nc.dram_tensor("dec", [BLOC, T, H], f32, kind="ExternalInput")
    ctx_d = nc.dram_tensor("ctx", [BLOC, S, C], f32, kind="ExternalInput")
    mask_d = nc.dram_tensor("mask", [BLOC, T, S], u8, kind="ExternalInput")
    wa_d = nc.dram_tensor("wa", [H, C], f32, kind="ExternalInput")
    out_d = nc.dram_tensor("out", [BLOC, T, C], f16, kind="ExternalOutput")
    with tile.TileContext(nc) as tc, ExitStack() as ctx:
        _emit(ctx, tc, out_d.ap(), dec_d.ap(), ctx_d.ap(), mask_d.ap(), wa_d.ap())
    nc.compile()
    _BUILT = nc
    return nc


def make_in_maps(decoder_output, context, mask, W_a):
    decoder_output = np.ascontiguousarray(np.asarray(decoder_output, dtype=np.float32))
    context = np.ascontiguousarray(np.asarray(context, dtype=np.float32))
    mask_u8 = np.ascontiguousarray(np.asarray(mask)).astype(np.uint8)
    W_a = np.ascontiguousarray(np.asarray(W_a, dtype=np.float32))
    in_maps = []
    for i in range(N_CORES):
        sl = slice(i * BLOC, (i + 1) * BLOC)
        in_maps.append(
            {
                "dec": decoder_output[sl],
                "ctx": context[sl],
                "mask": mask_u8[sl],
                "wa": W_a,
            }
        )
    return in_maps


def kernel(decoder_output, context, mask, W_a, **run_kwargs):
    nc = _build()
    in_maps = make_in_maps(decoder_output, context, mask, W_a)
    res = run_bass_kernel_spmd(nc, in_maps, core_ids=list(range(N_CORES)), **run_kwargs)
    out = np.concatenate([res.results[i]["out"] for i in range(N_CORES)], axis=0)
    return out.astype(np.float32)


if __name__ == "__main__":
    nc = _build()
    print("build + compile OK")


# revision 18
# speedup vs baseline: 1.1210x; 1.0271x over previous
"""Trainium2 Bass kernel for masked attention (nn_Attention_77704548319841).

Reference computation per batch b:
    CI     = einsum('sc,hc->hs', context[b], W_a)          # [H, S]
    scores = einsum('th,hs->ts', dec[b], CI)               # [T, S]
    scores = where(mask, -1e6, scores)
    attn   = softmax(scores, axis=-1)
    out[b] = einsum('ts,sc->tc', attn, context[b])         # [T, 2H]

Sharding: pure data parallel over batch (16 batches / 8 cores = 2 per core).
W_a is replicated.

Per-core pipeline (matmuls on TensorE, fp16 operands, f32 PSUM accum):
  mm1: CI[h,s]     = W_aT.T @ ctxT        (lhsT = W_a.T  [C,H], rhs = ctx.T [C,S])
  mm2: scores[t,s] = decT.T @ CI          (lhsT = dec.T  [H,T], rhs = CI    [H,S])
  softmax (free-dim S):  masked = mask*-1e6 + scores   (one DVE scalar_tensor_tensor)
                         exp    = Exp(masked - SHIFT), rowsum via ACT accum_out
                         attn   = exp * (1/rowsum)     (fp16, values in [0,1])
  mm3: out[t,c]    = attnT.T @ ctx        (lhsT = attn.T [S,T], rhs = ctx [S,C])

PE order: startup transposes + mm1(0), softmax-mm(0), mm1(1), mm3(0),
softmax-mm(1), mm3(1).  This order makes the sync-queue transposes'
readiness order equal their need order (ctxT b1 -> attnT b0 -> decT b1 ->
attnT b1), which matters because the tile scheduler orders queue work by
readiness.

Measured platform facts this schedule is built around:
  - plain big DMA ~300GB/s per queue; HBM ~358GB/s per core aggregate
  - HWDGE queues (scalar/sync) BLOCK THE ISSUING ENGINE until ring credits
    free (~1 outstanding big DMA); the gpsimd SWDGE queue does not block
  - xbar transpose ~1.27us per call regardless of size; ALL transposes must
    stay on ONE queue (concurrent transposes from two queues corrupt data);
    explicit dep-chains between them add ~1.8us each - rely on natural
    queue serialization instead
  - Pool-engine (gpsimd) casts are ~6x slower than DVE casts - never use
  - PE p-state: long idle gaps are followed by a half-speed ramp window

Queue plan:
  gpsimd SWDGE : ctx b0 ch0/ch1, ctx b1 ch0/ch1, dec b1, mask b1 (the
                 urgent stream + WAR-gated mask; non-blocking issues),
                 then odd-mt output stores
  scalar queue : wa, dec b0, mask b0 (issued while ACT idle), even stores
  sync queue   : ALL xbar transposes: ctxT b1, attnT b0, decT b1, attnT b1
  TensorE      : matmuls + startup transposes of W_a/ctx-b0/dec-b0 (PE idle)
  VectorE      : f32->fp16 casts, psum evictions, masking, attn scale (half)
  ScalarE      : exp (+fused rowsum), attn scale (half), half mm3 evictions

Output is stored as fp16 (halves store traffic; ~1e-4 extra rel err) and
upcast to f32 on the host after the gather.

Softmax normalization is mathematically exact w.r.t. the reference: a constant
shift (instead of rowmax) leaves softmax unchanged; masked entries get
exp(s - 1e6 - SHIFT) == 0, identical to the reference's where(-1e6) after its
own exp underflow.
"""

import numpy as np
from contextlib import ExitStack

import concourse.bass as bass
import concourse.tile as tile
from concourse import bacc, mybir
from concourse.masks import make_identity
from concourse.bass_utils import run_bass_kernel_spmd

B, T, S, H = 16, 1024, 1024, 512
C = 2 * H
N_CORES = 8
BLOC = B // N_CORES  # batches per core
P = 128
NT = T // P   # 8 t-tiles
NS = S // P   # 8 s-tiles
NH = H // P   # 4 h-tiles
NC_ = C // P  # 8 c-tiles
FD = 512      # matmul free-dim chunk
SHIFT = 100.0
NEG_BIG = -1.0e6

f32 = mybir.dt.float32
f16 = mybir.dt.float16
bf16 = mybir.dt.bfloat16
u8 = mybir.dt.uint8
AF = mybir.ActivationFunctionType
ALU = mybir.AluOpType


def _emit(ctx: ExitStack, tc: "tile.TileContext", out_d, dec_d, ctx_d, mask_d, wa_d):
    nc = tc.nc

    pw = ctx.enter_context(tc.tile_pool(name="pw", bufs=1))
    pin = ctx.enter_context(tc.tile_pool(name="pin", bufs=2))
    pstage = ctx.enter_context(tc.tile_pool(name="pstage", bufs=2))
    ptr = ctx.enter_context(tc.tile_pool(name="ptr", bufs=2))
    ptmp = ctx.enter_context(tc.tile_pool(name="ptmp", bufs=2))
    pout = ctx.enter_context(tc.tile_pool(name="pout", bufs=2))
    pstat = ctx.enter_context(tc.tile_pool(name="pstat", bufs=2))
    ppsum = ctx.enter_context(
        tc.tile_pool(name="ppsum", bufs=2, space=bass.MemorySpace.PSUM)
    )

    def transpose(dst, src):
        # xbar transposes MUST all go through one queue (concurrent transposes
        # on two HWDGE queues corrupt data - verified on HW).
        nc.sync.dma_start(dst, src, transpose=True)

    # ---- once-per-core constants --------------------------------------------
    bias_tile = pw.tile([P, 1], f32, tag="bias")
    nc.gpsimd.memset(bias_tile[:], -SHIFT)
    ident = pw.tile([P, P], f32, tag="ident")
    make_identity(nc, ident[:])
    identh = pw.tile([P, P], f16, tag="identh")
    nc.gpsimd.tensor_copy(identh[:], ident[:])
    wz = pw.tile([P, 256], f16, tag="wz")
    nc.gpsimd.memset(wz[:], 0.0)

    # waT[p, ct, h] = W_a.T[ct*128+p, h]
    waT = pw.tile([P, NC_, H], f16, tag="waT")

    def warm_mm(ps):
        # dummy matmul: keeps the PE HAM activity monitor in the warm state
        # (transpose-mode matmuls don't count as PE-busy for HAM)
        nc.tensor.matmul(ps[:, 0:256], wz[:, 0:P], wz[:], start=True, stop=True)

    def pe_transpose_block(dst_fp16, src_f16_tiles):
        """dst (fp16 [128, 4*128]) <- transposes of 4 fp16 [128,128] tiles.

        fp16 PE transposes run at 1 cycle/row (f32 takes 2) and halve the
        DVE eviction cost too."""
        # fp16 [P,1024] is byte-identical to the f32 [P,512] "psh" slot, so
        # it shares that tag (only the first 512 columns are used)
        pst = ppsum.tile([P, 2 * FD], f16, tag="psh", bufs=2, name="pst")
        for i, srct in enumerate(src_f16_tiles):
            nc.tensor.transpose(pst[:, i * P : (i + 1) * P], srct, identh[:])
        nc.vector.tensor_copy(dst_fp16, pst[:, 0:FD])

    def load_wa():
        # scalar queue first slot (ACT idle while ring-blocked)
        stg = pstage.tile([P, NH, C], f32, tag="wa_stg", bufs=1, name="wa_stg")
        nc.scalar.dma_start(stg[:], wa_d.rearrange("(a p) c -> p a c", p=P))
        return stg

    def wa_cast_transpose(stg):
        # shares the 8KB dec_f16 slot (disjoint lifetime: waT is consumed
        # before dec b0's cast lands)
        wf = pin.tile([P, NH, C], f16, tag="dec_f16", bufs=1, name="wa_f16")
        nc.vector.tensor_copy(wf[:], stg[:])
        for ct in range(NC_):
            pe_transpose_block(
                waT[:, ct, :],
                [wf[:, ht, ct * P : (ct + 1) * P] for ht in range(NH)],
            )

    # Per-batch tiles.
    ctxf = [None] * BLOC   # ctx fp16 natural  [p, st, c]
    ctxT = [None] * BLOC   # (lo, hi): half[p, ct, s_half] = ctx.T[ct*128+p, ...]
    decT = [None] * BLOC   # (lo, hi): half[p, kh, t_half] = dec.T[kh*128+p, ...]
    masku = [None] * BLOC  # mask u8 natural [p, tt, s]
    CI = [None] * BLOC     # CI fp16 [p, kh, s]
    attnT = [None] * BLOC  # 8 tiles [p, st, 128], one per t-tile

    ctx_stgs = {}

    def load_ctx_dma(b):
        """Two 2MB chunk loads on the gpsimd SWDGE queue."""
        cr = ctx_d[b].rearrange("(st p) c -> p st c", p=P)
        for half in range(2):
            sl = slice(half * 4, (half + 1) * 4)
            stg = pstage.tile([P, 4, C], f32, tag="ctx_stg", bufs=2,
                              name=f"ctx_stg{b}_{half}")
            nc.gpsimd.dma_start(stg[:], cr[:, sl, :])
            ctx_stgs[(b, half)] = stg

    def ctx_alloc(b):
        cf = pin.tile([P, NS, C], f16, tag="ctx_f16", bufs=2, name="ctx_f16")
        lo = ptr.tile([P, NC_, FD], f16, tag="ctxT_lo", bufs=1, name="ctxT_lo")
        hi = ptr.tile([P, NC_, FD], f16, tag="ctxT_hi", bufs=1, name="ctxT_hi")
        ctxf[b] = cf
        ctxT[b] = (lo, hi)

    def ctx_cast(b, half):
        """f32 staging -> fp16 natural layout (cf), on DVE."""
        stg = ctx_stgs[(b, half)]
        sl = slice(half * 4, (half + 1) * 4)
        nc.vector.tensor_copy(ctxf[b][:, sl, :], stg[:])

    def ctx_transpose_half_pe(b, half):
        """Startup path: PE transposes from the fp16 natural tile."""
        cf = ctxf[b]
        dst = ctxT[b][half]
        for ct in range(NC_):
            pe_transpose_block(
                dst[:, ct, :],
                [cf[:, half * 4 + j, ct * P : (ct + 1) * P] for j in range(4)],
            )

    def ctx_transpose_xbar(b):
        """Steady path: 8 xbar transposes from the fp16 natural tile."""
        for j8 in range(NS):
            half, j = divmod(j8, 4)
            dst = ctxT[b][half]
            transpose(dst[:, :, j * P : (j + 1) * P], ctxf[b][:, j8, :])

    def load_dec_dma(b, eng):
        stg = pstage.tile([P, NT, H], f32, tag=f"dec_stg{b}", bufs=1,
                          name=f"dec_stg{b}")
        eng.dma_start(stg[:], dec_d[b].rearrange("(tt p) h -> p tt h", p=P))
        return stg

    def dec_alloc(b):
        dlo = ptr.tile([P, NH, FD], f16, tag="decT_lo", bufs=1, name="decT_lo")
        dhi = ptr.tile([P, NH, FD], f16, tag="decT_hi", bufs=1, name="decT_hi")
        decT[b] = (dlo, dhi)

    def dec_transpose_pe(b, stg):
        df = pin.tile([P, NT, H], f16, tag="dec_f16", bufs=1, name="dec_f16")
        nc.vector.tensor_copy(df[:], stg[:])
        dlo, dhi = decT[b]
        for half in range(2):
            dst = dlo if half == 0 else dhi
            for kh in range(NH):
                pe_transpose_block(
                    dst[:, kh, :],
                    [df[:, half * 4 + tt, kh * P : (kh + 1) * P]
                     for tt in range(4)],
                )

    def dec_transpose_pe(b, stg):
        df = pin.tile([P, NT, H], f16, tag="dec_f16", bufs=1, name="dec_f16")
        nc.vector.tensor_copy(df[:], stg[:])
        dlo, dhi = decT[b]
        for half in range(2):
            dst = dlo if half == 0 else dhi
            for kh in range(NH):
                pe_transpose_block(
                    dst[:, kh, :],
                    [df[:, half * 4 + tt, kh * P : (kh + 1) * P]
                     for tt in range(4)],
                )

    def dec_cast_transpose_xbar(b, stg):
        df = pin.tile([P, NT, H], f16, tag="dec_f16", bufs=1, name="dec_f16")
        nc.vector.tensor_copy(df[:], stg[:])
        dlo, dhi = decT[b]
        for tt in range(NT):
            dst = dlo if tt < 4 else dhi
            transpose(dst[:, :, (tt % 4) * P : (tt % 4 + 1) * P], df[:, tt, :])

    def load_mask(b, eng):
        # shared buffer: b1's load WAR-waits on softmax(0)'s last mask read,
        # so it must go on the non-blocking gpsimd SWDGE queue
        mk = pin.tile([P, NT, S], u8, tag="mask", bufs=1, name=f"mask{b}")
        eng.dma_start(mk[:], mask_d[b].rearrange("(tt p) s -> p tt s", p=P))
        masku[b] = mk

    def mm1_begin(b):
        ci = ptr.tile([P, NH, S], f16, tag="CI", bufs=1, name="CI")
        CI[b] = ci

    def mm1_half(b, ns):
        ci = CI[b]
        rhs_t = ctxT[b][ns]
        for mh in range(NH):
            ps = ppsum.tile([P, FD], f32, tag="psh", bufs=2, name="psh")
            for ct in range(NC_):
                nc.tensor.matmul(
                    ps[:],
                    waT[:, ct, mh * P : (mh + 1) * P],
                    rhs_t[:, ct, :],
                    start=(ct == 0),
                    stop=(ct == NC_ - 1),
                )
            nc.vector.tensor_copy(ci[:, mh, ns * FD : (ns + 1) * FD], ps[:])

    def mm2_softmax(b, extra_slot=None):
        rs = pstat.tile([P, NT], f32, tag="rowsum")
        rr = pstat.tile([P, NT], f32, tag="rrec")
        aT = [
            ptr.tile([P, NS, P], f16, tag=f"attnT{mt}", bufs=1, name=f"attnT{mt}")
            for mt in range(NT)
        ]
        for mt in range(NT):
            ps = ppsum.tile([P, S], f32, tag="ps", bufs=3, name="ps")
            dth = decT[b][0] if mt < 4 else decT[b][1]
            for ns in range(2):
                for kh in range(NH):
                    nc.tensor.matmul(
                        ps[:, ns * FD : (ns + 1) * FD],
                        dth[:, kh, (mt % 4) * P : (mt % 4 + 1) * P],
                        CI[b][:, kh, ns * FD : (ns + 1) * FD],
                        start=(kh == 0),
                        stop=(kh == NH - 1),
                    )
            # masked = (mask * -1e6) + scores   (single DVE pass)
            sm = ptmp.tile([P, S], f32, tag="sm", bufs=2, name="sm")
            nc.vector.scalar_tensor_tensor(
                sm[:], masku[b][:, mt, :], NEG_BIG, ps[:], op0=ALU.mult, op1=ALU.add
            )
            # exp(masked - SHIFT) with fused rowsum
            ex = ptmp.tile([P, S], bf16, tag="ex", bufs=2, name="ex")
            nc.scalar.activation(
                ex[:], sm[:], AF.Exp, bias=bias_tile[:], scale=1.0,
                accum_out=rs[:, mt : mt + 1],
            )
            nc.vector.reciprocal(rr[:, mt : mt + 1], rs[:, mt : mt + 1])
            # attn = exp * (1/rowsum), fp16 in [0,1]; alternate DVE/ACT
            at = ptmp.tile([P, S], f16, tag="attn", bufs=3, name="attn")
            if mt % 2 == 0:
                nc.vector.tensor_scalar_mul(at[:], ex[:], rr[:, mt : mt + 1])
            else:
                nc.scalar.activation(
                    at[:], ex[:], AF.Copy, bias=0.0,
                    scale=rr[:, mt : mt + 1],
                )
            # attnT_mt[p, st, j] = attn[j, st*128+p]
            transpose(aT[mt][:], at[:])
            if extra_slot is not None:
                extra_slot(mt)
        attnT[b] = aT

    def mm3(b):
        for mt in range(NT):
            ps = ppsum.tile([P, S], f32, tag="ps", bufs=3, name="ps3")
            for nck in range(2):
                for ks in range(NS):
                    nc.tensor.matmul(
                        ps[:, nck * FD : (nck + 1) * FD],
                        attnT[b][mt][:, ks, :],
                        ctxf[b][:, ks, nck * FD : (nck + 1) * FD],
                        start=(ks == 0),
                        stop=(ks == NS - 1),
                    )
            ob = pout.tile([P, C], f16, tag="ob", bufs=2, name="ob")
            dst = out_d[b].rearrange("(tt p) c -> p tt c", p=P)[:, mt, :]
            if mt % 2 == 0:
                nc.scalar.copy(ob[:], ps[:])
                nc.scalar.dma_start(dst, ob[:])
            else:
                nc.vector.tensor_copy(ob[:], ps[:])
                nc.gpsimd.dma_start(dst, ob[:])

    # ---- schedule -----------------------------------------------------------
    wps = ppsum.tile([P, FD], f32, tag="psh", bufs=2, name="warm0")
    for _ in range(56):
        warm_mm(wps)

    # DMA issues. gpsimd SWDGE: ctx b0, ctx b1, dec b1, mask b1 (+odd
    # stores). scalar HWDGE: wa, dec b0, mask b0 (ACT idle while blocked).
    wa_stg = load_wa()
    load_ctx_dma(0)
    load_ctx_dma(1)
    dec_stg0 = load_dec_dma(0, nc.scalar)
    load_mask(0, nc.scalar)
    dec_stg1 = load_dec_dma(1, nc.gpsimd)
    load_mask(1, nc.gpsimd)

    # batch 0 startup: DVE casts feed fp16 PE transposes, mm1 slotted between
    ctx_alloc(0)
    dec_alloc(0)
    mm1_begin(0)
    ctx_cast(0, 0)                # DVE; also unblocks ctx-b1-ch0 (WAR on stg)
    ctx_transpose_half_pe(0, 0)   # PE: ctxT-lo (fp16)
    wa_cast_transpose(wa_stg)     # DVE cast + PE: waT (fp16)
    mm1_half(0, 0)                # PE: mm1 on lo while ch1 still landing
    ctx_cast(0, 1)
    ctx_transpose_half_pe(0, 1)   # PE: ctxT-hi
    mm1_half(0, 1)
    dec_transpose_pe(0, dec_stg0)

    # batch 1 ctx prep (lands early thanks to the dedicated gpsimd stream)
    ctx_alloc(1)
    ctx_cast(1, 0)
    ctx_cast(1, 1)
    ctx_transpose_xbar(1)

    mm2_softmax(0)
    dec_alloc(1)
    dec_cast_transpose_xbar(1, dec_stg1)
    mm1_begin(1)
    mm1_half(1, 0)
    mm1_half(1, 1)
    mm3(0)
    mm2_softmax(1)
    mm3(1)


_BUILT = None


def _build():
    global _BUILT
    if _BUILT is not None:
        return _BUILT
    nc = bacc.Bacc("TRN2", target_bir_lowering=False, debug=False)
    dec_d = nc.dram_tensor("dec", [BLOC, T, H], f32, kind="ExternalInput")
    ctx_d = nc.dram_tensor("ctx", [BLOC, S, C], f32, kind="ExternalInput")
    mask_d = nc.dram_tensor("mask", [BLOC, T, S], u8, kind="ExternalInput")
    wa_d = nc.dram_tensor("wa", [H, C], f32, kind="ExternalInput")
    out_d = nc.dram_tensor("out", [BLOC, T, C], f16, kind="ExternalOutput")
    with tile.TileContext(nc) as tc, ExitStack() as ctx:
        _emit(ctx, tc, out_d.ap(), dec_d.ap(), ctx_d.ap(), mask_d.ap(), wa_d.ap())
    nc.compile()
    _BUILT = nc
    return nc


def make_in_maps(decoder_output, context, mask, W_a):
    decoder_output = np.ascontiguousarray(np.asarray(decoder_output, dtype=np.float32))
    context = np.ascontiguousarray(np.asarray(context, dtype=np.float32))
    mask_u8 = np.ascontiguousarray(np.asarray(mask)).astype(np.uint8)
    W_a = np.ascontiguousarray(np.asarray(W_a, dtype=np.float32))
    in_maps = []
    for i in range(N_CORES):
        sl = slice(i * BLOC, (i + 1) * BLOC)
        in_maps.append(
            {
                "dec": decoder_output[sl],
                "ctx": context[sl],
                "mask": mask_u8[sl],
                "wa": W_a,
            }
        )
    return in_maps


def kernel(decoder_output, context, mask, W_a, **run_kwargs):
    nc = _build()
    in_maps = make_in_maps(decoder_output, context, mask, W_a)
    res = run_bass_kernel_spmd(nc, in_maps, core_ids=list(range(N_CORES)), **run_kwargs)
    out = np.concatenate([res.results[i]["out"] for i in range(N_CORES)], axis=0)
    return out.astype(np.float32)


if __name__ == "__main__":
    nc = _build()
    print("build + compile OK")
